# revision 29
# baseline (speedup 1.0000x reference)
"""Batch Conv1D kernel for Trainium2 — fp16, host-pre-transposed variant.

kernel() casts x/W to fp16 and pre-transposes x to [B, M, C, L] on the host
(layout prep, part of sharding). On-device per core:
  - One plain 512KB load per sequence: xt[c_local, cc, l] tiles.
  - Conv1D = 3 shifted matmuls over 2 C-chunks: 6 accumulating fp16 matmuls
    (1 cycle/row) per 128-position tile into fp32 PSUM.
  - Bias add + PSUM evacuation on VectorE, fp32 stores (one big store per
    sequence; ragged tail rows batched across 4 sequences).
Precision: fp16 inputs + fp32 accumulate measures 2.9e-4 absmax-relative
vs the fp32 reference on this problem's data.
"""

import numpy as np

B, M, L, C, F, K = 8, 32, 1024, 256, 256, 3  # noqa: E741
LOUT = L - K + 1  # 1022
SEQ = M
NT = L // 128
CC = C // 128
LAST = LOUT - 128 * (NT - 1)  # 126

_CACHE = {}


def _build_program():
    import concourse.bacc as bacc
    import concourse.mybir as mybir
    import concourse.tile as tile

    f16 = mybir.dt.float16
    f32 = mybir.dt.float32
    nc = bacc.Bacc(None, target_bir_lowering=False, debug=False)

    # x arrives pre-transposed: [SEQ, C, L]
    x_d = nc.dram_tensor("x", [SEQ, C, L], f16, kind="ExternalInput")
    w_d = nc.dram_tensor("W", [K, C, F], f16, kind="ExternalInput")
    b_d = nc.dram_tensor("b", [1, F], f32, kind="ExternalInput")
    y_d = nc.dram_tensor("y", [SEQ, LOUT, F], f32, kind="ExternalOutput")

    with tile.TileContext(nc) as tc:
        with (
            tc.tile_pool(name="const", bufs=1) as cpool,
            tc.tile_pool(name="xt", bufs=5) as xt_pool,
            tc.tile_pool(name="yout", bufs=4) as y_pool,
            tc.tile_pool(name="yrag", bufs=2) as yrag_pool,
            tc.tile_pool(name="psy", bufs=8, space="PSUM") as py_pool,
        ):
            # W on the scalar ring, first x chunks on the sync ring, in parallel
            w_sb = cpool.tile([128, K * CC, F], f16)
            nc.scalar.dma_start(
                out=w_sb, in_=w_d.rearrange("k (cc p) f -> p (k cc) f", cc=CC, p=128)
            )
            xt0 = xt_pool.tile([128, CC, L], f16, tag="xt", name="xt0")
            x_src0 = x_d[0].rearrange("(cc p) l -> p cc l", p=128)
            nc.sync.dma_start(out=xt0[:, :, 0:256], in_=x_src0[:, :, 0:256])
            nc.sync.dma_start(out=xt0[:, :, 256:512], in_=x_src0[:, :, 256:512])
            nc.sync.dma_start(out=xt0[:, :, 512:L], in_=x_src0[:, :, 512:L])

            b_row = cpool.tile([1, F], f32)
            nc.scalar.dma_start(out=b_row, in_=b_d[0:1, :])
            b_full = cpool.tile([128, F], f32)
            nc.gpsimd.partition_broadcast(b_full, b_row[0:1, :])

            AHEAD = 3

            def load_xt(s):
                xt = xt_pool.tile([128, CC, L], f16, tag="xt", name=f"xt{s}")
                nc.sync.dma_start(
                    out=xt, in_=x_d[s].rearrange("(cc p) l -> p cc l", p=128)
                )
                return xt

            xt_q = [xt0] + [load_xt(s) for s in range(1, AHEAD)]

            RG = 4  # ragged t7 rows batched across RG sequences into one DMA
            y_rag = None
            for s in range(SEQ):
                xt = xt_q.pop(0)
                if s + AHEAD < SEQ:
                    xt_q.append(load_xt(s + AHEAD))
                if s % RG == 0:
                    y_rag = yrag_pool.tile([LAST, RG, F], f32, tag="y_rag")

                # Conv matmuls: y[l0+m, f] = sum_{k, cc} xT[:, l0+k+m].T @ W[k, cc]
                y_sb = y_pool.tile([128, NT - 1, F], f32)
                t_order = range(NT) if s < SEQ - 1 else [NT - 1, *range(NT - 1)]
                for t in t_order:
                    mt = 128 if t < NT - 1 else LAST
                    ps_y = py_pool.tile([128, F], f32, tag="ps_y")
                    idx = 0
                    for k in range(K):
                        for cc in range(CC):
                            nc.tensor.matmul(
                                ps_y[0:mt, :],
                                xt[:, cc, t * 128 + k : t * 128 + k + mt],
                                w_sb[:, k * CC + cc, :],
                                start=(idx == 0),
                                stop=(idx == 5),
                            )
                            idx += 1
                    dst = y_sb[0:mt, t, :] if t < NT - 1 else y_rag[:, s % RG, :]
                    nc.vector.tensor_add(dst, ps_y[0:mt, :], b_full[0:mt, :])

                y_r = y_d[s, 0 : 128 * (NT - 1)].rearrange("(t p) f -> p t f", p=128)
                if s < SEQ - 1:
                    nc.scalar.dma_start(out=y_r, in_=y_sb)
                else:
                    # drain the last sequence in pieces behind its matmuls
                    nc.scalar.dma_start(out=y_r[:, 0:3, :], in_=y_sb[:, 0:3, :])
                    nc.scalar.dma_start(out=y_r[:, 3:5, :], in_=y_sb[:, 3:5, :])
                    nc.scalar.dma_start(out=y_r[:, 5:7, :], in_=y_sb[:, 5:7, :])
                if s % RG == RG - 1:
                    nc.scalar.dma_start(
                        out=y_d[s - RG + 1 : s + 1, 128 * (NT - 1) : LOUT, :].rearrange(
                            "sg p f -> p sg f"
                        ),
                        in_=y_rag,
                    )

    nc.compile()
    return nc


def _get_program():
    if "nc" not in _CACHE:
        _CACHE["nc"] = _build_program()
    return _CACHE["nc"]


def kernel(x: np.ndarray, W: np.ndarray, b: np.ndarray) -> np.ndarray:
    from concourse.bass_utils import run_bass_kernel_spmd

    nc = _get_program()
    # host-side layout prep: cast to fp16 and put C on the partition axis
    xt_host = np.ascontiguousarray(
        np.asarray(x).astype(np.float16).transpose(0, 1, 3, 2)
    )  # [B, M, C, L]
    W16 = np.ascontiguousarray(np.asarray(W), dtype=np.float16)
    b2 = np.ascontiguousarray(np.asarray(b), dtype=np.float32).reshape(1, F)
    in_maps = [{"x": xt_host[i], "W": W16, "b": b2} for i in range(B)]
    res = run_bass_kernel_spmd(nc, in_maps, core_ids=list(range(B)))
    return np.stack([r["y"] for r in res.results], axis=0)


# revision 41
# speedup vs baseline: 1.0160x; 1.0160x over previous
"""Batch Conv1D kernel for Trainium2 — fp16, host-pre-transposed variant.

kernel() casts x/W to fp16 and pre-transposes x to [B, M, C, L] on the host
(layout prep, part of sharding). On-device per core:
  - One plain 512KB load per sequence: xt[c_local, cc, l] tiles.
  - Conv1D = 3 shifted matmuls over 2 C-chunks: 6 accumulating fp16 matmuls
    (1 cycle/row) per 128-position tile into fp32 PSUM.
  - Bias add + PSUM evacuation on VectorE, fp32 stores (one big store per
    sequence; ragged tail rows batched across 4 sequences).
Precision: fp16 inputs + fp32 accumulate measures 2.9e-4 absmax-relative
vs the fp32 reference on this problem's data.
"""

import numpy as np

B, M, L, C, F, K = 8, 32, 1024, 256, 256, 3  # noqa: E741
LOUT = L - K + 1  # 1022
SEQ = M
NT = L // 128
CC = C // 128
LAST = LOUT - 128 * (NT - 1)  # 126

_CACHE = {}


def _build_program():
    import concourse.bacc as bacc
    import concourse.mybir as mybir
    import concourse.tile as tile

    f16 = mybir.dt.float16
    f32 = mybir.dt.float32
    nc = bacc.Bacc(None, target_bir_lowering=False, debug=False)

    # x arrives pre-transposed: [SEQ, C, L]
    x_d = nc.dram_tensor("x", [SEQ, C, L], f16, kind="ExternalInput")
    w_d = nc.dram_tensor("W", [K, C, F], f16, kind="ExternalInput")
    b_d = nc.dram_tensor("b", [1, F], f32, kind="ExternalInput")
    y_d = nc.dram_tensor("y", [SEQ, LOUT, F], f32, kind="ExternalOutput")

    with tile.TileContext(nc) as tc:
        with (
            tc.tile_pool(name="const", bufs=1) as cpool,
            tc.tile_pool(name="xt", bufs=5) as xt_pool,
            tc.tile_pool(name="yout", bufs=4) as y_pool,
            tc.tile_pool(name="yrag", bufs=2) as yrag_pool,
            tc.tile_pool(name="psy", bufs=8, space="PSUM") as py_pool,
        ):
            # W on the scalar ring, first x chunks on the sync ring, in parallel
            w_sb = cpool.tile([128, K * CC, F], f16)
            nc.scalar.dma_start(
                out=w_sb, in_=w_d.rearrange("k (cc p) f -> p (k cc) f", cc=CC, p=128)
            )
            xt0 = xt_pool.tile([128, CC, L], f16, tag="xt", name="xt0")
            x_src0 = x_d[0].rearrange("(cc p) l -> p cc l", p=128)
            nc.sync.dma_start(out=xt0[:, :, 0:132], in_=x_src0[:, :, 0:132])
            nc.sync.dma_start(out=xt0[:, :, 132:512], in_=x_src0[:, :, 132:512])
            nc.sync.dma_start(out=xt0[:, :, 512:L], in_=x_src0[:, :, 512:L])

            b_row = cpool.tile([1, F], f32)
            nc.scalar.dma_start(out=b_row, in_=b_d[0:1, :])
            b_full = cpool.tile([128, F], f32)
            nc.gpsimd.partition_broadcast(b_full, b_row[0:1, :])

            AHEAD = 3

            def load_xt(s):
                xt = xt_pool.tile([128, CC, L], f16, tag="xt", name=f"xt{s}")
                nc.sync.dma_start(
                    out=xt, in_=x_d[s].rearrange("(cc p) l -> p cc l", p=128)
                )
                return xt

            xt_q = [xt0] + [load_xt(s) for s in range(1, AHEAD)]

            RG = 4  # ragged t7 rows batched across RG sequences into one DMA
            y_rag = None
            for s in range(SEQ):
                xt = xt_q.pop(0)
                if s + AHEAD < SEQ:
                    xt_q.append(load_xt(s + AHEAD))
                if s % RG == 0:
                    y_rag = yrag_pool.tile([LAST, RG, F], f32, tag="y_rag")

                # Conv matmuls: y[l0+m, f] = sum_{k, cc} xT[:, l0+k+m].T @ W[k, cc]
                y_sb = y_pool.tile([128, NT - 1, F], f32)
                t_order = range(NT) if s < SEQ - 1 else [NT - 1, *range(NT - 1)]
                for t in t_order:
                    mt = 128 if t < NT - 1 else LAST
                    ps_y = py_pool.tile([128, F], f32, tag="ps_y")
                    idx = 0
                    for k in range(K):
                        for cc in range(CC):
                            nc.tensor.matmul(
                                ps_y[0:mt, :],
                                xt[:, cc, t * 128 + k : t * 128 + k + mt],
                                w_sb[:, k * CC + cc, :],
                                start=(idx == 0),
                                stop=(idx == 5),
                            )
                            idx += 1
                    dst = y_sb[0:mt, t, :] if t < NT - 1 else y_rag[:, s % RG, :]
                    nc.vector.tensor_add(dst, ps_y[0:mt, :], b_full[0:mt, :])

                y_r = y_d[s, 0 : 128 * (NT - 1)].rearrange("(t p) f -> p t f", p=128)
                if s < SEQ - 1:
                    nc.scalar.dma_start(out=y_r, in_=y_sb)
                else:
                    # drain the last sequence in pieces behind its matmuls
                    nc.scalar.dma_start(out=y_r[:, 0:3, :], in_=y_sb[:, 0:3, :])
                    nc.scalar.dma_start(out=y_r[:, 3:5, :], in_=y_sb[:, 3:5, :])
                    nc.scalar.dma_start(out=y_r[:, 5:7, :], in_=y_sb[:, 5:7, :])
                if s % RG == RG - 1:
                    nc.scalar.dma_start(
                        out=y_d[s - RG + 1 : s + 1, 128 * (NT - 1) : LOUT, :].rearrange(
                            "sg p f -> p sg f"
                        ),
                        in_=y_rag,
                    )

    nc.compile()
    return nc


def _get_program():
    if "nc" not in _CACHE:
        _CACHE["nc"] = _build_program()
    return _CACHE["nc"]


def kernel(x: np.ndarray, W: np.ndarray, b: np.ndarray) -> np.ndarray:
    from concourse.bass_utils import run_bass_kernel_spmd

    nc = _get_program()
    # host-side layout prep: cast to fp16 and put C on the partition axis
    xt_host = np.ascontiguousarray(
        np.asarray(x).astype(np.float16).transpose(0, 1, 3, 2)
    )  # [B, M, C, L]
    W16 = np.ascontiguousarray(np.asarray(W), dtype=np.float16)
    b2 = np.ascontiguousarray(np.asarray(b), dtype=np.float32).reshape(1, F)
    in_maps = [{"x": xt_host[i], "W": W16, "b": b2} for i in range(B)]
    res = run_bass_kernel_spmd(nc, in_maps, core_ids=list(range(B)))
    return np.stack([r["y"] for r in res.results], axis=0)
